# revision 37
# baseline (speedup 1.0000x reference)
"""Trainium2 Bass kernel: 8-expert top-2 MoE layer (SwiGLU experts).

Sharding: paired expert parallelism across 8 NeuronCores. Experts are
paired heaviest-with-lightest; each pair lands on two cores that both
process BOTH experts' full token sets over HALF of the intermediate dim
(an exact decomposition: gate/up split along their output dim, the down
projection's partial contractions summed on the host). Per-core work is
(max heavy load + max light load)/2 token-equivalents instead of the max
expert load. The host performs the router (exact fp64 softmax/top-2,
shipped as per-token combine weights) and the token dispatch/combine.
The FFN runs in bf16 with fp32 PSUM accumulation.

All device inputs are pre-packed on the host into the exact SBUF tile
layout (partition-major, fully contiguous per partition) so every DMA
runs at full queue rate — the DGE queues degrade badly on short
gather lines (~90 GB/s at 256 B vs ~260 GB/s contiguous on the SWDGE).
Wave-0 gate/up weights ship as four 256-feature units hand-scheduled
across the SWDGE (gpsimd) and both HWDGE queues (sync/scalar) in PE
consumption order, so the first matmul issues ~14 us in instead of ~20.
The last wave ends on a 128-token block whose output DMAs ride the
(by then idle) SWDGE queue to shorten the kernel tail.

Self-contained: hardcodes all shapes from the problem spec.
"""

import os

import numpy as np

# Problem constants
H = 1024  # hidden dim
I = 4096  # intermediate dim
E = 8  # experts
P = 128  # SBUF partitions
IH = I // 2  # intermediate features per core (half of I)

# Tiling constants
TB = 512  # max tokens per block (matmul moving free dim)
IS = 1024  # intermediate features resident per weight wave
N_SUPER = IH // IS  # weight waves per segment (= 2)
IT = IS // P  # i-tiles per wave
HO = H // P  # h chunks (contraction tiles)
HH = H // 512  # output column halves for the down projection
IU = 256  # wave-0 weight unit width (i features)
NU = IS // IU  # wave-0 units per wave (= 4)


def _blocks(Tc, first=384, last=0):
    """Token blocks for one segment. The first block is small-ish (the
    wave-0 weights race the PE; a 384-token first block starts the PE
    earlier than a 512 one while still consuming weights slower than
    the DMA queues deliver them). An optional small `last` block
    shortens the kernel tail on the final wave."""
    assert Tc % P == 0 and Tc >= 256
    sizes = []
    rem = Tc
    if last and rem >= first + last + 256:
        rem -= last
    else:
        last = 0
    if rem >= first + 256:
        sizes.append(first)
        rem -= first
    while rem > 512:
        sizes.append(512)
        rem -= 512
    if rem:
        sizes.append(rem)
    if last:
        sizes.append(last)
    blocks = []
    t = 0
    for tb in sizes:
        blocks.append((t, tb))
        t += tb
    return blocks


def build_moe(TCA: int, TCB: int):
    """Per-core program: segments A/B of TCA/TCB tokens, half-I each."""
    import concourse.bass as bass  # noqa: F401
    import concourse.mybir as mybir
    import concourse.tile as tile
    from concourse import bacc

    f32 = mybir.dt.float32
    bf16 = mybir.dt.bfloat16
    Alu = mybir.AluOpType
    Act = mybir.ActivationFunctionType

    nc = bacc.Bacc(
        "TRN2", target_bir_lowering=False, debug=False, num_devices=8
    )

    segs = {}
    for s, Tc in (("a", TCA), ("b", TCB)):
        seg = {
            "Tc": Tc,
            "blocks": _blocks(Tc, last=(128 if s == "b" else 0)),
            "NW": Tc // P,
            "wal": nc.dram_tensor(f"wal{s}", [P, Tc // P], f32, kind="ExternalInput").ap(),
            "wd": nc.dram_tensor(f"wd{s}", [P, IH // P, H], bf16, kind="ExternalInput").ap(),
            # Each wave writes its own bf16 partial-sum buffer (no
            # read-modify-write accumulate anywhere; the host adds the
            # two). bf16 halves cost ~1e-3 extra relative error and
            # halve the wave-0 output write traffic.
            "out": nc.dram_tensor(f"out{s}", [Tc, H], bf16, kind="ExternalOutput").ap(),
            "out2": nc.dram_tensor(f"out2{s}", [Tc, H], bf16, kind="ExternalOutput").ap(),
        }
        # Per-block x tensors, pre-packed [P, HO, tb].
        seg["x"] = [
            nc.dram_tensor(f"x{s}{bi}", [P, HO, tb], bf16, kind="ExternalInput").ap()
            for bi, (_, tb) in enumerate(seg["blocks"])
        ]
        segs[s] = seg
    # Segment A wave 0: four 256-feature units per side (startup race).
    sa = segs["a"]
    sa["wg0u"] = [
        nc.dram_tensor(f"wg0au{u}", [P, HO, IU], bf16, kind="ExternalInput").ap()
        for u in range(NU)
    ]
    sa["wu0u"] = [
        nc.dram_tensor(f"wu0au{u}", [P, HO, IU], bf16, kind="ExternalInput").ap()
        for u in range(NU)
    ]
    # All other waves: one packed [P, HO, IS] tensor per side per wave.
    sa["wg1"] = nc.dram_tensor("wg1a", [P, HO, IS], bf16, kind="ExternalInput").ap()
    sa["wu1"] = nc.dram_tensor("wu1a", [P, HO, IS], bf16, kind="ExternalInput").ap()
    sb = segs["b"]
    for sup in range(N_SUPER):
        sb[f"wg{sup}"] = nc.dram_tensor(
            f"wg{sup}b", [P, HO, IS], bf16, kind="ExternalInput"
        ).ap()
        sb[f"wu{sup}"] = nc.dram_tensor(
            f"wu{sup}b", [P, HO, IS], bf16, kind="ExternalInput"
        ).ap()

    with tile.TileContext(nc) as tc:
        with (
            tc.tile_pool(name="singles", bufs=1) as singles,
            tc.tile_pool(name="xres", bufs=1) as xres,
            tc.tile_pool(name="w0", bufs=1) as w0pool,
            tc.tile_pool(name="weights", bufs=2) as wpool,
            tc.tile_pool(name="hp", bufs=2) as hpool,
            tc.tile_pool(name="ep", bufs=3) as epool,
            tc.tile_pool(name="pgu", bufs=2, space="PSUM") as pgu,
            tc.tile_pool(name="pout", bufs=3, space="PSUM") as pout,
        ):
            # x tiles are SHARED between the two segments (union of their
            # block-size multisets) and reloaded at each wave; the WAR
            # dependencies through the tile framework schedule each
            # reload during the preceding wave.
            from collections import Counter

            need = Counter()
            for seg in segs.values():
                c = Counter(tb for _, tb in seg["blocks"])
                for sz, n in c.items():
                    need[sz] = max(need[sz], n)
            xtiles = {
                sz: [
                    xres.tile([P, HO, sz], bf16, tag=f"xt{sz}_{k}", name=f"xt{sz}_{k}")
                    for k in range(n)
                ]
                for sz, n in need.items()
            }
            for s, seg in segs.items():
                seg["wal_sb"] = singles.tile(
                    [P, seg["NW"]], f32, tag=f"wal{s}", name=f"wal{s}"
                )
                used = Counter()
                seg["x_sb"] = []
                for _, tb in seg["blocks"]:
                    seg["x_sb"].append(xtiles[tb][used[tb]])
                    used[tb] += 1



            # Waves: (segment, super) in execution order. Both A-waves
            # run first so A's x loads once; B's x loads exactly once,
            # interleaved into wave a1 as each shared tile is freed.
            waves = [("a", 0), ("a", 1), ("b", 0), ("b", 1)]
            sa_, sb_ = segs["a"], segs["b"]
            a_ids = {id(t): i for i, t in enumerate(sa_["x_sb"])}
            b_load, prologue_b = {}, []
            for bi_ in range(len(sb_["blocks"])):
                t_ = sb_["x_sb"][bi_]
                src_ = sb_["x"][bi_]
                if id(t_) in a_ids:
                    b_load.setdefault(a_ids[id(t_)], []).append((t_, src_))
                else:
                    prologue_b.append((t_, src_))
            # Pending down-projection groups carry ACROSS wave boundaries:
            # the previous wave's last block's down groups interleave into
            # the next wave's first block's i-tile loop (the closure binds
            # that wave's wd/wal/output), so the PE stream never hits a
            # flush-then-stall pattern at a wave transition. Only the very
            # last wave flushes at the end.
            pendings = []
            for wi, (s, sup) in enumerate(waves):
                seg = segs[s]
                blocks = seg["blocks"]
                x_sb = seg["x_sb"]
                wal_sb = seg["wal_sb"]
                wd_q = [
                    wpool.tile([P, 2, H], bf16, tag=f"wd_q{q}", name=f"wd_q{q}")
                    for q in range(4)
                ]
                wd_src = [
                    seg["wd"][:, sup * IT + 2 * q : sup * IT + 2 * q + 2, :]
                    for q in range(4)
                ]
                if wi == 0:
                    # Wave-0 gate/up weights race the PE. The early phase
                    # is AGGREGATE-bandwidth bound (~330 GB/s across all
                    # DMA rings), so what matters is that every queue's
                    # HEAD transfer is on the critical path: x block 0 on
                    # the SWDGE, the gate units on sync, the up units on
                    # scalar — each queue in PE consumption order.
                    w0u = {
                        (side, u): w0pool.tile(
                            [P, HO, IU], bf16, tag=f"w0_{side}{u}", name=f"w0_{side}{u}"
                        )
                        for side in ("g", "u")
                        for u in range(NU)
                    }
                    # The three head transfers are split in two along the
                    # (contiguous) ho dim: the DGE fair-shares ring
                    # bandwidth per ENTRY, so two entries double the
                    # critical tiles' share and the first matmul issues
                    # a few us earlier.
                    nc.gpsimd.dma_start(
                        x_sb[0][:, : HO // 2, :], seg["x"][0][:, : HO // 2, :]
                    )
                    nc.gpsimd.dma_start(
                        x_sb[0][:, HO // 2 :, :], seg["x"][0][:, HO // 2 :, :]
                    )
                    for u in range(NU):
                        if u == 0:
                            for h0, h1 in ((0, HO // 2), (HO // 2, HO)):
                                nc.sync.dma_start(
                                    w0u[("g", u)][:, h0:h1, :],
                                    seg["wg0u"][u][:, h0:h1, :],
                                )
                                nc.scalar.dma_start(
                                    w0u[("u", u)][:, h0:h1, :],
                                    seg["wu0u"][u][:, h0:h1, :],
                                )
                        else:
                            nc.sync.dma_start(w0u[("g", u)], seg["wg0u"][u])
                            nc.scalar.dma_start(w0u[("u", u)], seg["wu0u"][u])

                    def wgt(it, w0u=w0u):
                        u, r = divmod(it, IU // P)
                        return w0u[("g", u)][:, :, r * P : (r + 1) * P]

                    def wut(it, w0u=w0u):
                        u, r = divmod(it, IU // P)
                        return w0u[("u", u)][:, :, r * P : (r + 1) * P]

                    # The rest of the prologue, in need order: the next
                    # x block, combine weights, the wave's down weights
                    # (four quarters so the first down group only waits
                    # on the first; the last two ride the HWDGE queues
                    # behind the units), remaining x blocks, B's own.
                    if len(blocks) > 1:
                        nc.gpsimd.dma_start(x_sb[1], seg["x"][1])
                    nc.gpsimd.dma_start(wal_sb, seg["wal"])
                    nc.gpsimd.dma_start(wd_q[0], wd_src[0])
                    nc.gpsimd.dma_start(wd_q[1], wd_src[1])
                    nc.sync.dma_start(wd_q[2], wd_src[2])
                    nc.scalar.dma_start(wd_q[3], wd_src[3])
                    # x blocks 2+ and segment B's prologue are emitted
                    # later (after block 1's eviction entries in the
                    # GPSIMD stream) so their submission is gated past
                    # the startup-critical DMA window.
                else:
                    # Later waves' weights load via the GPSIMD stream,
                    # placed AFTER the previous wave's first eviction
                    # entries: the engine processes its stream in order,
                    # so these submissions are semaphore-gated until the
                    # previous wave is well underway — keeping the
                    # startup window's aggregate DMA bandwidth for the
                    # critical wave-0 tiles.
                    wg_sb = wpool.tile([P, HO, IS], bf16, tag="wg", name="wg_sb")
                    nc.gpsimd.dma_start(wg_sb, seg["wg1"] if s == "a" else seg[f"wg{sup}"])
                    wu_sb = wpool.tile([P, HO, IS], bf16, tag="wu", name="wu_sb")
                    nc.gpsimd.dma_start(wu_sb, seg["wu1"] if s == "a" else seg[f"wu{sup}"])

                    def wgt(it, wg_sb=wg_sb):
                        return wg_sb[:, :, it * P : (it + 1) * P]

                    def wut(it, wu_sb=wu_sb):
                        return wu_sb[:, :, it * P : (it + 1) * P]

                    for q, eng in ((0, nc.gpsimd), (1, nc.gpsimd),
                                   (2, nc.gpsimd), (3, nc.gpsimd)):
                        eng.dma_start(wd_q[q], wd_src[q])
                wd_sb = wd_q

                def down_group(t0, h_sb, grp, flush, seg=seg, sup=sup, wd_sb=wd_sb,
                               wal_sb=wal_sb, last_wave=(wi == len(waves) - 1)):
                    # One (token-subtile, output-half) group of the down
                    # projection, back to token-partition layout, scaled by
                    # the combine weight at PSUM eviction. Wave 0 writes
                    # f32; wave 1 writes a separate bf16 buffer on the
                    # HWDGE queues so the SWDGE queue is quiet long before
                    # the kernel tail — except the very last groups, which
                    # ride the (by then idle) SWDGE to cut the tail.
                    tsub, hh = divmod(grp, HH)
                    col = t0 // P + tsub
                    r0 = t0 + tsub * P
                    ops = pout.tile([P, 512], f32, tag="o", name="o")
                    for it in range(IT):
                        nc.tensor.matmul(
                            ops,
                            lhsT=h_sb[:, it, tsub * P : (tsub + 1) * P],
                            rhs=wd_sb[it // 2][:, it % 2, hh * 512 : (hh + 1) * 512],
                            start=(it == 0),
                            stop=(it == IT - 1),
                        )
                    if sup == N_SUPER - 1:
                        oev2 = epool.tile([P, 512], bf16, tag="oev2", name="ov2")
                        nc.vector.tensor_scalar_mul(
                            oev2, ops, wal_sb[:, col : col + 1]
                        )
                        if last_wave and flush:
                            eng = nc.gpsimd
                        elif last_wave:
                            eng = nc.sync if (col + hh) % 2 == 0 else nc.scalar
                        else:
                            eng = nc.gpsimd
                        eng.dma_start(
                            seg["out2"][r0 : r0 + P, hh * 512 : (hh + 1) * 512],
                            oev2,
                        )
                    else:
                        oev = epool.tile([P, 512], bf16, tag="oev", name="oev")
                        nc.vector.tensor_scalar_mul(
                            oev, ops, wal_sb[:, col : col + 1]
                        )
                        nc.gpsimd.dma_start(
                            seg["out"][r0 : r0 + P, hh * 512 : (hh + 1) * 512],
                            oev,
                        )

                for bi, (t0, tb) in enumerate(blocks):
                    tsn = tb // P
                    # Expert FFN for this (i-chunk, token block):
                    # hT[i, t] = silu(Wg.T x)[i, t] * (Wu.T x)[i, t]
                    h_sb = hpool.tile([P, IT, TB], bf16, tag="h", name="h")[:, :, :tb]
                    dgn = tsn * HH

                    def gu_mm(ps, wt, bi=bi):
                        for ho in range(HO):
                            nc.tensor.matmul(
                                ps,
                                lhsT=wt[:, ho, :],
                                rhs=x_sb[bi][:, ho, :],
                                start=(ho == 0),
                                stop=(ho == HO - 1),
                            )

                    # The kernel's very first i-tiles run both GATE
                    # phases (it0+it1, which only need the first gate
                    # unit) before any UP phase: the scalar engine's
                    # DMA queue submits ~1.3 us later than sync's (it
                    # loads the activation table first), so the first
                    # up unit lands late — this reorder hides it.
                    kickoff = 2 if (wi == 0 and bi == 0) else 0
                    kick = []
                    for it in range(kickoff):
                        gps = pgu.tile([P, TB], f32, tag="g", name="g")[:, :tb]
                        gu_mm(gps, wgt(it))
                        kick.append(gps)
                    for it in range(kickoff):
                        ups = pgu.tile([P, TB], f32, tag="u", name="u")[:, :tb]
                        gu_mm(ups, wut(it))
                        gs = epool.tile([P, TB], f32, tag="gs", name="gs")[:, :tb]
                        nc.scalar.activation(gs, kick[it], Act.Silu)
                        nc.vector.tensor_tensor(
                            h_sb[:, it, :], gs, ups, op=Alu.mult
                        )
                    for it in range(kickoff, IT):
                        gps = pgu.tile([P, TB], f32, tag="g", name="g")[:, :tb]
                        ups = pgu.tile([P, TB], f32, tag="u", name="u")[:, :tb]
                        gu_mm(gps, wgt(it))
                        gu_mm(ups, wut(it))
                        gs = epool.tile([P, TB], f32, tag="gs", name="gs")[:, :tb]
                        nc.scalar.activation(gs, gps, Act.Silu)
                        nc.vector.tensor_tensor(
                            h_sb[:, it, :], gs, ups, op=Alu.mult
                        )
                        if pendings:
                            p_t0, p_h, p_dgn, p_dg = pendings[0]
                            for grp in range(
                                it * p_dgn // IT, (it + 1) * p_dgn // IT
                            ):
                                p_dg(p_t0, p_h, grp, False)

                    if wi == 0 and bi == min(1, len(blocks) - 1):
                        for bj in range(2, len(blocks)):
                            nc.gpsimd.dma_start(x_sb[bj], seg["x"][bj])
                        nc.gpsimd.dma_start(segs["b"]["wal_sb"], segs["b"]["wal"])
                        for t_, src_ in prologue_b:
                            nc.gpsimd.dma_start(t_, src_)
                    if wi == 1:
                        for t_, src_ in b_load.get(bi, ()):
                            nc.gpsimd.dma_start(t_, src_)
                    if pendings:
                        pendings.pop(0)
                    pendings.append((t0, h_sb, dgn, down_group))
            for p_t0, p_h, p_dgn, p_dg in pendings:
                for grp in range(p_dgn):
                    p_dg(p_t0, p_h, grp, True)

    nc.compile()
    return nc


def _run_spmd(nc, in_maps, trace):
    from concourse import bass_utils

    if trace:
        try:
            res = bass_utils.run_bass_kernel_spmd(
                nc, in_maps, core_ids=list(range(E)), trace=True
            )
            if res.exec_time_ns is not None:
                print(f"HW exec time: {res.exec_time_ns} ns")
            return res
        except Exception as exc:  # fall back to an untraced run
            print(f"traced run failed ({exc!r}); retrying without trace")
    return bass_utils.run_bass_kernel_spmd(
        nc, in_maps, core_ids=list(range(E)), trace=False
    )


def prepare(hidden_states, gate_proj_w, gate_weights, up_weights, down_weights):
    """Host router + paired dispatch; returns (nc, in_maps, combine_fn)."""
    import ml_dtypes

    bf16 = ml_dtypes.bfloat16
    x = np.ascontiguousarray(hidden_states, dtype=np.float32)
    gpw = np.ascontiguousarray(gate_proj_w, dtype=np.float32)
    T = x.shape[0]

    # Router in fp64: logits -> softmax -> top-2 (stable ties like
    # jax.lax.top_k) -> renormalized combine weights.
    logits = x.astype(np.float64) @ gpw.astype(np.float64).T  # [T, E]
    pr = np.exp(logits - logits.max(axis=1, keepdims=True))
    pr /= pr.sum(axis=1, keepdims=True)
    top2 = np.argsort(-pr, axis=1, kind="stable")[:, :2]
    pv = np.take_along_axis(pr, top2, axis=1)
    wts = (pv / pv.sum(axis=1, keepdims=True)).astype(np.float32)  # [T, 2]

    idx = [np.nonzero((top2 == e).any(axis=1))[0] for e in range(E)]
    cnt = np.array([len(ix) for ix in idx])

    # Pair heaviest with lightest: segment A = 4 heaviest experts,
    # segment B = 4 lightest, pair rank k of A with rank -k of B.
    order = np.argsort(-cnt, kind="stable")
    pairs = [(int(order[k]), int(order[E - 1 - k])) for k in range(E // 2)]

    def pad128(n):
        return max(512, ((n + P - 1) // P) * P)

    TCA = pad128(max(cnt[a] for a, _ in pairs))
    TCB = pad128(max(cnt[b] for _, b in pairs))
    blocks_a = _blocks(TCA)
    blocks_b = _blocks(TCB, last=128)

    nc = build_moe(TCA, TCB)

    def pack_w(w):  # [H, n] -> [P, HO, n], h = ho*P + p
        n = w.shape[1]
        return np.ascontiguousarray(
            w.reshape(HO, P, n).transpose(1, 0, 2)
        ).astype(bf16)

    def seg_inputs(s, e, Tc, half, blocks):
        n_e = len(idx[e])
        xTe = np.zeros((H, Tc), dtype=np.float32)
        if n_e:
            xTe[:, :n_e] = x[idx[e]].T
        we = np.zeros((Tc,), dtype=np.float32)
        if n_e:
            we[:n_e] = np.where(
                top2[idx[e], 0] == e, wts[idx[e], 0], wts[idx[e], 1]
            )
        lo, hi = half * IH, (half + 1) * IH
        wgh = gate_weights[e][:, lo:hi]
        wuh = up_weights[e][:, lo:hi]
        wdh = down_weights[e][lo:hi, :]  # [IH, H]
        m = {
            f"wal{s}": np.ascontiguousarray(we.reshape(Tc // P, P).T),
            f"wd{s}": np.ascontiguousarray(
                wdh.reshape(IH // P, P, H).transpose(1, 0, 2)
            ).astype(bf16),
        }
        for bi, (t0, tb) in enumerate(blocks):
            m[f"x{s}{bi}"] = pack_w(xTe[:, t0 : t0 + tb])
        if s == "a":
            for u in range(NU):
                m[f"wg0au{u}"] = pack_w(wgh[:, u * IU : (u + 1) * IU])
                m[f"wu0au{u}"] = pack_w(wuh[:, u * IU : (u + 1) * IU])
            m["wg1a"] = pack_w(wgh[:, IS:])
            m["wu1a"] = pack_w(wuh[:, IS:])
        else:
            for sup in range(N_SUPER):
                m[f"wg{sup}b"] = pack_w(wgh[:, sup * IS : (sup + 1) * IS])
                m[f"wu{sup}b"] = pack_w(wuh[:, sup * IS : (sup + 1) * IS])
        return m

    in_maps = []
    core_expert = []  # (expert_a, expert_b) per core
    for a, b in pairs:
        for half in range(2):
            m = seg_inputs("a", a, TCA, half, blocks_a)
            m.update(seg_inputs("b", b, TCB, half, blocks_b))
            in_maps.append(m)
            core_expert.append((a, b))

    def combine(results):
        out = np.zeros((T, H), dtype=np.float32)
        for core, (a, b) in enumerate(core_expert):
            r = results[core]
            n_a = len(idx[a])
            if n_a:
                out[idx[a]] += (
                    r["outa"][:n_a].astype(np.float32)
                    + r["out2a"][:n_a].astype(np.float32)
                )
            n_b = len(idx[b])
            if n_b:
                out[idx[b]] += (
                    r["outb"][:n_b].astype(np.float32)
                    + r["out2b"][:n_b].astype(np.float32)
                )
        return out

    return nc, in_maps, combine


def kernel(hidden_states, gate_proj_w, gate_weights, up_weights, down_weights):
    trace = os.environ.get("MOE_TRACE", "0") == "1"
    nc, in_maps, combine = prepare(
        hidden_states, gate_proj_w, gate_weights, up_weights, down_weights
    )
    res = _run_spmd(nc, in_maps, trace)
    out = combine(res.results)
    if not np.isfinite(out).all():
        # Transient device corruption (rare; observed only on profiled
        # runs that closely follow other device activity). Retry a few
        # times untraced with a pause for the device to settle.
        import time

        for _ in range(3):
            time.sleep(2.0)
            res = _run_spmd(nc, in_maps, False)
            out = combine(res.results)
            if np.isfinite(out).all():
                break
    return out
